# revision 1
# baseline (speedup 1.0000x reference)
"""Trainium2 Bass kernel for a ViT attention block (LN->MHA+relpos->LN->MLP).

Contract: kernel(**inputs) takes the FULL unsharded inputs, shards batch
across 8 NeuronCores (4 items per core), runs one SPMD Bass program, and
gathers the full [32, 577, 768] fp32 output.

Design notes
- All matmuls in bf16 with fp32 PSUM accumulation.
- LayerNorm gamma/beta are folded into the following matmul weights on the
  host; only the per-token (x - mean) * rstd runs on device.
- Activations flow channel-major ([C, tokens]) for matmul RHS; normalized
  activations are transposed via one DMA-xbar transpose through DRAM.
- Attention per (b, h): S^T[m, n] = k^T.T @ q^T; relative-position bias is
  accumulated into the S PSUM tile by an identity-weight matmul; exp runs on
  the scalar engine straight out of PSUM; P^T @ [v | 1] yields O^T plus the
  softmax denominator (ones-column trick); the reciprocal row is
  partition-broadcast on GPSIMD for the normalize multiply.
- Tokens are padded 577 -> 640 per batch item so m-chunking is 5x128. Padded
  key columns are zero and their rel-pos bias is -30, so exp ~= 0.
"""

import sys

if '/opt/trn_rl_repo' not in sys.path:
    sys.path.insert(0, '/opt/trn_rl_repo')

from contextlib import ExitStack

import numpy as np
import ml_dtypes

import concourse.bass as bass  # noqa: F401
import concourse.tile as tile
import concourse.mybir as mybir
from concourse import bacc, bass_utils
from concourse.masks import make_identity

BF16 = ml_dtypes.bfloat16
F32 = np.float32

B = 32
N = 577
C = 768
NH = 12
HD = 64
MLP = 3072
EPS = 1e-6
SCALE = HD ** (-0.5)

N_CORES = 8
BPC = B // N_CORES          # 4 batch items per core
NPAD = 640                  # per-item padded token count (5 * 128)
TOK = BPC * NPAD            # 2560 padded tokens per core
NCH = TOK // 128            # 20 token chunks
KC = C // 128               # 6 contraction chunks for dim 768
MC = MLP // 128             # 24 chunks for MLP dim
MCHUNK = NPAD // 128        # 5 m-chunks per batch item
F32T = mybir.dt.float32
BF16T = mybir.dt.bfloat16
AF = mybir.ActivationFunctionType
OP = mybir.AluOpType

SPLITS_N = [(0, 512), (512, 65)]   # 577-wide outputs (PSUM bank = 512 fp32)
SPLITS_C = [(0, 512), (512, 256)]  # 768-wide outputs (bank-aligned)

ZERO_ALL_SLABS = False  # sim-only: defeat the pool-slot zero-persistence


def _ln_stats(nc, pool, xt, eps_sb):
    """Per-token mean/rstd for a [128, C] fp32 chunk -> (mean_ap, rstd_ap)."""
    st = pool.tile([128, 2, 6], F32T, tag="bnst")
    nc.vector.bn_stats(st[:, 0, :], xt[:, 0:C // 2])
    nc.vector.bn_stats(st[:, 1, :], xt[:, C // 2:C])
    mv = pool.tile([128, 2], F32T, tag="bnmv")
    nc.vector.bn_aggr(mv[:], st[:])
    sd = pool.tile([128, 1], F32T, tag="sd")
    nc.scalar.activation(sd[:], mv[:, 1:2], AF.Sqrt, bias=eps_sb[:, 0:1])
    rstd = pool.tile([128, 1], F32T, tag="rstd")
    nc.vector.reciprocal(rstd[:], sd[:])
    return mv, rstd


def build_program(nc):
    dt = mybir.dt

    x_d = nc.dram_tensor("x", [TOK, C], dt.float32, kind="ExternalInput")
    xb_d = nc.dram_tensor("xb", [TOK, C], dt.float32, kind="ExternalInput")
    wqk_d = nc.dram_tensor("wqkT", [C, 2 * C], dt.bfloat16, kind="ExternalInput")
    bqk_d = nc.dram_tensor("bias_qk", [2 * C], dt.float32, kind="ExternalInput")
    wv_d = nc.dram_tensor("wvT", [C, C], dt.bfloat16, kind="ExternalInput")
    bv_d = nc.dram_tensor("bias_v", [C], dt.bfloat16, kind="ExternalInput")
    wp_d = nc.dram_tensor("wprojT", [C, C], dt.bfloat16, kind="ExternalInput")
    bp_d = nc.dram_tensor("bias_proj", [C], dt.bfloat16, kind="ExternalInput")
    w1_d = nc.dram_tensor("w1T", [C, MLP], dt.bfloat16, kind="ExternalInput")
    b1_d = nc.dram_tensor("bias_fc1", [MLP], dt.float32, kind="ExternalInput")
    w2_d = nc.dram_tensor("w2T", [MLP, C], dt.bfloat16, kind="ExternalInput")
    b2_d = nc.dram_tensor("bias_fc2", [C], dt.bfloat16, kind="ExternalInput")
    rpb_d = nc.dram_tensor("rpbT", [NH, NPAD, N], dt.bfloat16,
                           kind="ExternalInput")
    out_d = nc.dram_tensor("out", [TOK, C], dt.float32, kind="ExternalOutput")

    xh_d = nc.dram_tensor("xh_scratch", [TOK, C], dt.bfloat16)
    xh2_d = nc.dram_tensor("xh2_scratch", [TOK, C], dt.bfloat16)
    x2_d = nc.dram_tensor("x2_scratch", [TOK, C], dt.float32)
    rec_d = nc.dram_tensor("rec_scratch", [BPC, NH, N], dt.float32)

    x_ap = x_d.ap().rearrange("(c p) d -> p c d", p=128)      # [128, 20, 768]
    xb_ap = xb_d.ap().rearrange("(c p) d -> p c d", p=128)
    xh_ap = xh_d.ap().rearrange("(c p) d -> p c d", p=128)
    xh2_ap = xh2_d.ap().rearrange("(c p) d -> p c d", p=128)
    x2_ap = x2_d.ap().rearrange("(c p) d -> p c d", p=128)
    out_ap = out_d.ap().rearrange("(c p) d -> p c d", p=128)

    with tile.TileContext(nc) as tc, ExitStack() as ctx:
        persist = ctx.enter_context(tc.tile_pool(name="persist", bufs=1))
        psum = ctx.enter_context(tc.tile_pool(name="psum", bufs=4, space="PSUM"))

        ident = persist.tile([128, 128], BF16T, tag="ident")
        make_identity(nc, ident[:])
        eps_sb = persist.tile([128, 1], F32T, tag="eps")
        nc.vector.memset(eps_sb[:], EPS)
        bqk_sb = persist.tile([128, 12], F32T, tag="bqk")
        nc.sync.dma_start(bqk_sb[:], bqk_d.ap().rearrange("(m p) -> p m", p=128))
        bv_sb = persist.tile([128, C], BF16T, tag="bv")
        bvsrc = bv_d.ap()
        nc.sync.dma_start(bv_sb[:], bass.AP(
            tensor=bvsrc.tensor, offset=bvsrc.offset,
            ap=[[0, 128]] + list(bvsrc.ap)))
        bp_sb = persist.tile([1, C], BF16T, tag="bp")
        nc.sync.dma_start(bp_sb[:], bp_d.ap().rearrange("(o c) -> o c", o=1))
        bfc1_sb = persist.tile([128, MC], F32T, tag="bfc1")
        nc.sync.dma_start(bfc1_sb[:], b1_d.ap().rearrange("(m p) -> p m", p=128))
        bfc2_sb = persist.tile([128, C], BF16T, tag="bfc2")
        b2src = b2_d.ap()
        nc.sync.dma_start(bfc2_sb[:], bass.AP(
            tensor=b2src.tensor, offset=b2src.offset,
            ap=[[0, 128]] + list(b2src.ap)))

        # ---------- attention-superphase scope ----------
        abc_ctx = ExitStack()
        ap_w = abc_ctx.enter_context(tc.tile_pool(name="attnw", bufs=1))
        wp_sb = ap_w.tile([128, KC, C], BF16T, tag="wp")
        nc.sync.dma_start(wp_sb[:], wp_d.ap().rearrange("(k p) c -> p k c", p=128))
        wqk_sb = ap_w.tile([128, KC, 2 * C], BF16T, tag="wqk")
        nc.sync.dma_start(
            wqk_sb[:], wqk_d.ap().rearrange("(k p) c -> p k c", p=128))
        wv_sb = ap_w.tile([128, KC, C], BF16T, tag="wv")
        nc.sync.dma_start(
            wv_sb[:], wv_d.ap().rearrange("(k p) c -> p k c", p=128))

        # per-b double-buffered big slabs
        qkp = abc_ctx.enter_context(tc.tile_pool(name="qkp", bufs=2))
        kzp = abc_ctx.enter_context(tc.tile_pool(name="kzp", bufs=2))
        vp = abc_ctx.enter_context(tc.tile_pool(name="vp", bufs=2))
        xhp = abc_ctx.enter_context(tc.tile_pool(name="xhp", bufs=2))
        rpbp = abc_ctx.enter_context(tc.tile_pool(name="rpb", bufs=2))
        ptp = abc_ctx.enter_context(tc.tile_pool(name="pt", bufs=2))
        osbp = abc_ctx.enter_context(tc.tile_pool(name="osb", bufs=12))
        smallp = abc_ctx.enter_context(tc.tile_pool(name="attnsmall", bufs=3))
        rbp = abc_ctx.enter_context(tc.tile_pool(name="rbp", bufs=2))
        ck = abc_ctx.enter_context(tc.tile_pool(name="lnck", bufs=2))
        ck2 = abc_ctx.enter_context(tc.tile_pool(name="projck", bufs=2))

        def emit_ln1_b(b):
            """LN1 for item b\'s 5 token chunks -> xh_dram, then transpose
            into a fresh per-b xhT slab."""
            for i in range(b * MCHUNK, (b + 1) * MCHUNK):
                xt = ck.tile([128, C], F32T, tag="xt")
                nc.sync.dma_start(xt[:], x_ap[:, i, :])
                mv, rstd = _ln_stats(nc, ck, xt, eps_sb)
                xh_t = ck.tile([128, C], BF16T, tag="xh")
                nc.vector.tensor_scalar(
                    out=xh_t[:], in0=xt[:], scalar1=mv[:, 0:1],
                    scalar2=rstd[:, 0:1], op0=OP.subtract, op1=OP.mult)
                nc.sync.dma_start(xh_ap[:, i, :], xh_t[:])
            xhT = xhp.tile([128, KC, NPAD], BF16T, tag="xhT")
            nc.sync.dma_start_transpose(
                xhT[:], xh_d.ap()[b * NPAD:(b + 1) * NPAD, :])
            return xhT

        def emit_s_chunk(qkT, kz, hp, h01, pt, mc, e_tile):
            """One S chunk at full K=128: kz holds this head's k in its own
            64 rows and zeros in the other head's rows, so k^T.T @ q_pair
            gives exactly this head's scores at full stream width."""
            h = 2 * hp + h01
            mw = 128 if mc < MCHUNK - 1 else N - 4 * 128  # 65
            sps = psum.tile([128, 768], F32T, tag="ps")
            for (lo, w) in SPLITS_N:
                nc.tensor.matmul(
                    sps[:mw, lo:lo + w],
                    lhsT=kz[:, h, mc * 128: mc * 128 + mw],
                    rhs=qkT[:, hp, lo:lo + w],
                    start=True, stop=False)
                nc.tensor.matmul(
                    sps[:mw, lo:lo + w], lhsT=ident[0:mw, 0:mw],
                    rhs=e_tile[0:mw, mc, lo:lo + w],
                    start=False, stop=True)
            nc.scalar.activation(pt[:mw, mc, :], sps[:mw, 0:N], AF.Exp)

        class PvCtx:
            """Pending P^T @ [v|1] for one head, drained a few matmuls at a
            time between the next head's S chunks."""

            def __init__(self, pt, v_sb, h, den12, o_list):
                self.pt, self.v_sb, self.h = pt, v_sb, h
                self.den12, self.o_list = den12, o_list
                self.pv = psum.tile([128, 768], F32T, tag="ps")
                self.mms = [(lo, w, mc) for (lo, w) in SPLITS_N
                            for mc in range(MCHUNK)]
                self.pos = 0

            def drain(self, k):
                end = min(self.pos + k, len(self.mms))
                for (lo, w, mc) in self.mms[self.pos:end]:
                    mw = 128 if mc < MCHUNK - 1 else N - 4 * 128
                    nc.tensor.matmul(
                        self.pv[:, lo:lo + w],
                        lhsT=self.v_sb[0:mw, mc,
                                       self.h * 66: self.h * 66 + 128],
                        rhs=self.pt[0:mw, mc, lo:lo + w],
                        start=(mc == 0), stop=(mc == MCHUNK - 1))
                self.pos = end
                if self.pos == len(self.mms):
                    dd = smallp.tile([1, N], F32T, tag="dd")
                    nc.scalar.activation(dd[:], self.pv[64:65, 0:N],
                                         AF.Identity, bias=0.0)
                    nc.sync.dma_start(self.den12[self.h:self.h + 1, :], dd[:])
                    o_sb = osbp.tile([64, N], BF16T, tag="osb")
                    nc.scalar.activation(o_sb[:], self.pv[0:64, 0:N],
                                         AF.Identity, bias=0.0)
                    self.o_list[self.h] = o_sb
                    self.pv = None
                    return True
                return False

            def finish(self):
                while self.pv is not None:
                    self.drain(4)

        def emit_proj_chunk(kz, b, ic, half):
            """One half (512 or 256 cols) of proj+residual+LN2 for chunk ic.
            half=0 emits the 512 split; half=1 emits the 256 split plus the
            residual/LN2 tail."""
            i = b * MCHUNK + ic
            if half == 0:
                ps = psum.tile([128, 768], F32T, tag="ps")
                proj_ps[ic] = ps
            else:
                ps = proj_ps.pop(ic)
            (lo, w) = SPLITS_C[half]
            for cc in range(KC):
                nc.tensor.matmul(
                    ps[:, lo:lo + w],
                    lhsT=kz[:, 2 * cc, ic * 128:(ic + 1) * 128],
                    rhs=wp_sb[:, cc, lo:lo + w],
                    start=(cc == 0), stop=(cc == KC - 1))
            if half == 0:
                return
            xt = ck2.tile([128, C], F32T, tag="xt2")
            nc.sync.dma_start(xt[:], xb_ap[:, i, :])
            x2t = ck2.tile([128, C], F32T, tag="x2t")
            nc.vector.tensor_tensor(x2t[:], ps[:, 0:C], xt[:], OP.add)
            nc.sync.dma_start(x2_ap[:, i, :], x2t[:])
            mv, rstd = _ln_stats(nc, ck2, x2t, eps_sb)
            xh2t = ck2.tile([128, C], BF16T, tag="xh2")
            nc.vector.tensor_scalar(
                out=xh2t[:], in0=x2t[:], scalar1=mv[:, 0:1],
                scalar2=rstd[:, 0:1], op0=OP.subtract, op1=OP.mult)
            nc.sync.dma_start(xh2_ap[:, i, :], xh2t[:])

        proj_ps = {}

        def finalize_steps(b, kz, den12, o_list):
            """Secondary-step closures: reciprocal, 12 OT writes, 10 proj
            half-chunks for item b."""
            steps = []

            def recip_step():
                rec12 = smallp.tile([12, N], F32T, tag="rec")
                nc.vector.reciprocal(rec12[:], den12[:])
                nc.sync.dma_start(rec_d.ap()[b], rec12[:])
            steps.append(recip_step)

            def mult_step(h):
                base = 64 * (h % 2)
                rb = rbp.tile([64, N], F32T, tag="rb")
                rsrc = rec_d.ap()[b, h]
                nc.sync.dma_start(rb[:], bass.AP(
                    tensor=rsrc.tensor, offset=rsrc.offset,
                    ap=[[0, 64]] + list(rsrc.ap)))
                nc.vector.tensor_tensor(
                    kz[base:base + 64, 2 * (h // 2), 0:N], o_list[h][:],
                    rb[:], OP.mult)
            for h in range(NH):
                steps.append(lambda h=h: mult_step(h))
            for ic in range(MCHUNK):
                for half in range(2):
                    steps.append(
                        lambda ic=ic, half=half: emit_proj_chunk(
                            kz, b, ic, half))
            return steps

        def qkv_steps(b, xhT):  # noqa: b used in alloc_step
            """Secondary-step closures computing q/k/v for item b into fresh
            per-b slabs. Returns (steps, result_cell)."""
            cell = {}

            def alloc_step():
                qkT = qkp.tile([128, 6, NPAD], BF16T, tag="qkT")
                kz = kzp.tile([128, 12, NPAD], BF16T, tag="kz")
                cell['kz'] = kz
                v_sb = vp.tile([128, MCHUNK, NH * 66 + 62], BF16T, tag="v")
                if b < 2 or ZERO_ALL_SLABS:
                    # pool slots alternate; the constant regions are never
                    # overwritten by data, so zeroing the first two slabs
                    # covers all four items
                    nc.vector.memset(kz[:], 0.0)
                    nc.vector.memset(
                        v_sb[:, :, 0:NH * 66].rearrange(
                            "p m (h e) -> p m h e", e=66)[:, :, :, 64:66], 1.0)
                    nc.vector.memset(v_sb[:, :, NH * 66:], 0.0)
                cell['qkT'], cell['v'] = qkT, v_sb

            qk_ps = {}

            def qk_step(oc, half):
                if half == 0:
                    ps = psum.tile([128, 768], F32T, tag="ps")
                    qk_ps[oc] = ps
                else:
                    ps = qk_ps.pop(oc)
                (lo, w) = SPLITS_N[half]
                for kc in range(KC):
                    nc.tensor.matmul(
                        ps[:, lo:lo + w],
                        lhsT=wqk_sb[:, kc, oc * 128:(oc + 1) * 128],
                        rhs=xhT[:, kc, lo:lo + w],
                        start=(kc == 0), stop=(kc == KC - 1))
                if half == 1:
                    if oc < 6:
                        nc.vector.tensor_scalar(
                            out=cell['qkT'][:, oc, 0:N], in0=ps[:, 0:N],
                            scalar1=bqk_sb[:, oc:oc + 1], scalar2=None,
                            op0=OP.add)
                    else:
                        p2 = oc - 6
                        nc.vector.tensor_scalar(
                            out=cell['kz'][0:64, 2 * p2, 0:N],
                            in0=ps[0:64, 0:N],
                            scalar1=bqk_sb[0:64, oc:oc + 1], scalar2=None,
                            op0=OP.add)
                        nc.vector.tensor_scalar(
                            out=cell['kz'][64:128, 2 * p2 + 1, 0:N],
                            in0=ps[64:128, 0:N],
                            scalar1=bqk_sb[64:128, oc:oc + 1], scalar2=None,
                            op0=OP.add)

            v_ps = {}

            def v_step(mc, half):
                mw = 128 if mc < MCHUNK - 1 else N - 4 * 128
                if half == 0:
                    ps = psum.tile([128, 768], F32T, tag="ps")
                    v_ps[mc] = ps
                else:
                    ps = v_ps.pop(mc)
                (lo, w) = SPLITS_C[half]
                for kc in range(KC):
                    nc.tensor.matmul(
                        ps[:mw, lo:lo + w],
                        lhsT=xhT[:, kc, mc * 128: mc * 128 + mw],
                        rhs=wv_sb[:, kc, lo:lo + w],
                        start=(kc == 0), stop=(kc == KC - 1))
                if half == 1:
                    nc.vector.tensor_tensor(
                        cell['v'][0:mw, mc, 0:NH * 66].rearrange(
                            "p (h e) -> p h e", e=66)[:, :, 0:64],
                        ps[0:mw, 0:768].rearrange("p (h e) -> p h e", h=NH),
                        bv_sb[0:mw, :].rearrange("p (h e) -> p h e", h=NH),
                        OP.add)

            steps = [alloc_step]
            for oc in range(12):
                steps.append(lambda oc=oc: qk_step(oc, 0))
                steps.append(lambda oc=oc: qk_step(oc, 1))
            for mc in range(MCHUNK):
                steps.append(lambda mc=mc: v_step(mc, 0))
                steps.append(lambda mc=mc: v_step(mc, 1))
            return steps, cell

        # ---------------- main pipelined loop ----------------
        # Prologue: item 0 qkv emitted wholesale.
        xhT_cur = emit_ln1_b(0)
        q_steps, q_cell = qkv_steps(0, xhT_cur)
        for s in q_steps:
            s()
        cur = (q_cell['qkT'], q_cell['kz'], q_cell['v'])
        states = {}
        pending = [None, None]
        for b in range(BPC):
            qkT, kz, v_sb = cur
            den12 = smallp.tile([12, N], F32T, tag="den")
            o_list = [None] * NH
            states[b] = (den12, o_list)
            if b + 1 < BPC:
                xhT_next = emit_ln1_b(b + 1)
            sec = []
            if b - 1 in states:
                sec += finalize_steps(b - 1, prev_kz, *states.pop(b - 1))
            if b + 1 < BPC:
                q_steps, q_cell = qkv_steps(b + 1, xhT_next)
                sec += q_steps
            sec_i = 0
            for head_idx, (hp, h01) in enumerate(
                    (hp, h01) for hp in range(6) for h01 in range(2)):
                h = 2 * hp + h01
                if h01 == 0:
                    e_tiles = []
                    for hh in (h, h + 1):
                        rt = rpbp.tile([128, MCHUNK, N], BF16T, tag="rpb")
                        nc.sync.dma_start(
                            rt[:],
                            rpb_d.ap()[hh].rearrange("(m p) n -> p m n", p=128))
                        e_tiles.append(rt)
                pt = ptp.tile([128, MCHUNK, N], BF16T, tag="pt")
                for mc in range(MCHUNK):
                    emit_s_chunk(qkT, kz, hp, h01, pt, mc, e_tiles[h01])
                    if pending[0] is not None:
                        if pending[0].drain(2 if mc < MCHUNK - 1 else 4):
                            pending[0] = None
                    if head_idx >= 1 and sec_i < len(sec):
                        sec[sec_i]()
                        sec_i += 1
                if pending[0] is not None:
                    pending[0].finish()
                    pending[0] = None
                pending[0] = PvCtx(pt, v_sb, h, den12, o_list)
            # drain remaining secondary steps for this b
            while sec_i < len(sec):
                sec[sec_i]()
                sec_i += 1
            prev_kz = kz
            if b + 1 < BPC:
                cur = (q_cell['qkT'], q_cell['kz'], q_cell['v'])
        if pending[0] is not None:
            pending[0].finish()
            pending[0] = None
        for s in finalize_steps(BPC - 1, prev_kz, *states.pop(BPC - 1)):
            s()

        abc_ctx.close()

        # ================= MLP =================
        with ExitStack() as mctx:
            mlpp = mctx.enter_context(tc.tile_pool(name="mlp", bufs=1))
            w1_sb = mlpp.tile([128, KC, MLP], BF16T, tag="w1")
            w1_src = w1_d.ap().rearrange("(k p) c -> p k c", p=128)
            for kc in range(KC):
                nc.sync.dma_start(w1_sb[:, kc, :], w1_src[:, kc, :])
            w2_sb = mlpp.tile([128, MC, C], BF16T, tag="w2")
            w2_src = w2_d.ap().rearrange("(k p) c -> p k c", p=128)
            for mc8 in range(4):
                nc.sync.dma_start(w2_sb[:, mc8 * 6:(mc8 + 1) * 6, :],
                                  w2_src[:, mc8 * 6:(mc8 + 1) * 6, :])
            xh2T = mlpp.tile([128, 2, KC, TOK // 2], BF16T, tag="xh2T")
            for half in range(2):
                nc.sync.dma_start_transpose(
                    xh2T[:, half], xh2_d.ap()[half * (TOK // 2):
                                              (half + 1) * (TOK // 2), :])

            mtp = mctx.enter_context(tc.tile_pool(name="mt", bufs=2))
            ck3 = mctx.enter_context(tc.tile_pool(name="mlpck", bufs=3))
            NB = 256
            for nb in range(TOK // NB):
                mt = mtp.tile([128, MC, NB], BF16T, tag="mt")
                for mc in range(MC):
                    mps = psum.tile([128, 768], F32T, tag="ps")
                    half, loc = nb // 5, nb % 5
                    for kc in range(KC):
                        nc.tensor.matmul(
                            mps[:, 0:NB],
                            lhsT=w1_sb[:, kc, mc * 128:(mc + 1) * 128],
                            rhs=xh2T[:, half, kc, loc * NB:(loc + 1) * NB],
                            start=(kc == 0), stop=(kc == KC - 1))
                    nc.scalar.activation(mt[:, mc, :], mps[:, 0:NB], AF.Gelu,
                                         bias=bfc1_sb[:, mc:mc + 1])
                for ns in range(NB // 128):
                    i = nb * (NB // 128) + ns
                    fps = psum.tile([128, 768], F32T, tag="ps")
                    for (lo, w) in SPLITS_C:
                        for mc in range(MC):
                            nc.tensor.matmul(
                                fps[:, lo:lo + w],
                                lhsT=mt[:, mc, ns * 128:(ns + 1) * 128],
                                rhs=w2_sb[:, mc, lo:lo + w],
                                start=(mc == 0), stop=(mc == MC - 1))
                    xf = ck3.tile([128, C], F32T, tag="xf")
                    nc.sync.dma_start(xf[:], x2_ap[:, i, :])
                    ot = ck3.tile([128, C], F32T, tag="ot")
                    nc.vector.tensor_tensor(ot[:], fps[:, 0:C], xf[:], OP.add)
                    nc.vector.tensor_tensor(ot[:], ot[:], bfc2_sb[:], OP.add)
                    nc.sync.dma_start(out_ap[:, i, :], ot[:])


def host_prep(inputs):
    """Fold layernorms/biases/scale into weights; build per-core input maps."""
    x = np.asarray(inputs['x'], F32)
    qkv_w = np.asarray(inputs['qkv_w'], F32)
    g1 = np.asarray(inputs['norm1_g'], F32)
    b1 = np.asarray(inputs['norm1_b'], F32)
    q_bias = np.asarray(inputs['q_bias'], F32)
    v_bias = np.asarray(inputs['v_bias'], F32)
    rpb_table = np.asarray(inputs['rpb_table'], F32)
    rel_index = np.asarray(inputs['rel_index'])
    proj_w = np.asarray(inputs['proj_w'], F32)
    proj_b = np.asarray(inputs['proj_b'], F32)
    g2 = np.asarray(inputs['norm2_g'], F32)
    b2 = np.asarray(inputs['norm2_b'], F32)
    fc1_w = np.asarray(inputs['fc1_w'], F32)
    fc1_b = np.asarray(inputs['fc1_b'], F32)
    fc2_w = np.asarray(inputs['fc2_w'], F32)
    fc2_b = np.asarray(inputs['fc2_b'], F32)

    Wq = qkv_w[0:C] * g1[None, :] * SCALE
    bias_q = (qkv_w[0:C] @ b1 + q_bias) * SCALE
    Wk = qkv_w[C:2 * C] * g1[None, :]
    bias_k = qkv_w[C:2 * C] @ b1
    Wv = qkv_w[2 * C:] * g1[None, :]
    bias_v = qkv_w[2 * C:] @ b1 + v_bias

    wqkT = np.ascontiguousarray(np.concatenate([Wq, Wk], 0).T).astype(BF16)
    bias_qk = np.concatenate([bias_q, bias_k]).astype(F32)
    wvT = np.ascontiguousarray(Wv.T).astype(BF16)
    wprojT = np.ascontiguousarray(proj_w.T).astype(BF16)
    w1T = np.ascontiguousarray((fc1_w * g2[None, :]).T).astype(BF16)
    bias_fc1 = (fc1_w @ b2 + fc1_b).astype(F32)
    w2T = np.ascontiguousarray(fc2_w.T).astype(BF16)

    rpb = rpb_table[rel_index]                     # [N, N, NH]
    rpbT = np.full((NH, NPAD, N), -30.0, F32)      # pad rows -> exp ~= 0
    rpbT[:, :N, :] = rpb.transpose(2, 1, 0)        # rpbT[h, m, n] = rpb[n, m, h]
    rpbT = rpbT.astype(BF16)

    shared = dict(
        wqkT=wqkT, bias_qk=bias_qk, wvT=wvT, bias_v=bias_v.astype(BF16),
        wprojT=wprojT, bias_proj=proj_b.astype(BF16),
        w1T=w1T, bias_fc1=bias_fc1, w2T=w2T, bias_fc2=fc2_b.astype(BF16),
        rpbT=rpbT)

    xpad = np.zeros((B, NPAD, C), F32)
    xpad[:, :N, :] = x
    xbpad = xpad + proj_b[None, None, :].astype(F32)
    in_maps = []
    for core in range(N_CORES):
        xi = xpad[core * BPC:(core + 1) * BPC].reshape(TOK, C)
        xbi = xbpad[core * BPC:(core + 1) * BPC].reshape(TOK, C)
        m = dict(shared)
        m['x'] = np.ascontiguousarray(xi)
        m['xb'] = np.ascontiguousarray(xbi)
        in_maps.append(m)
    return in_maps


def build_bass():
    nc = bacc.Bacc("TRN2", target_bir_lowering=False, debug=False,
                   num_devices=N_CORES)
    build_program(nc)
    nc.compile()
    return nc


def gather_output(results):
    out = np.zeros((B, N, C), F32)
    for core in range(N_CORES):
        o = results[core]["out"].reshape(BPC, NPAD, C)
        out[core * BPC:(core + 1) * BPC] = o[:, :N, :]
    return out


def kernel(**inputs):
    in_maps = host_prep(inputs)
    nc = build_bass()
    res = bass_utils.run_bass_kernel_spmd(nc, in_maps,
                                          core_ids=list(range(N_CORES)))
    return gather_output(res.results)

